# revision 1
# baseline (speedup 1.0000x reference)
"""nn_GRUBlock Trainium2 kernel: y = GRU2(gelu(GRU1(x))).

Self-contained: builds a Bass/Tile program, shards batch B=16 across 8
NeuronCores (B=2 per core), runs via run_bass_kernel_spmd, gathers the
full output.

Per-core program (both GRU layers sequential over T in chunks):
  - input projections as 128-tile GEMMs (moving N = S*NB timestep-batch cols)
  - recurrence: per step, 48 [128x128] fp16 matmuls (w_hh.T stationary,
    h.T moving N=NB) into PSUM; gates on DVE/ACT in [128, *] layout
  - hidden state kept in (t, j, b) transposed layout so h' feeds the next
    step's matmul directly, no transposes anywhere
  - matmul operands fp16 (fp32 PSUM accumulate + fp32 gates):
    end-to-end rel err vs fp32 reference ~6e-4
"""

from contextlib import ExitStack

import numpy as np

B, T, DIN, H = 16, 4096, 512, 512
N_CORES = 8
NB = B // N_CORES      # batch per core
S = 128                # chunk (steps)
U = 8                  # step-loop unroll inside tc.For_i

_CACHE = {}


def _build(T_, S_, NB_, U_):
    import concourse.bacc as bacc
    import concourse.bass as bass
    import concourse.tile as tile
    from concourse import mybir

    F32 = mybir.dt.float32
    F16 = mybir.dt.float16
    AF = mybir.ActivationFunctionType
    ALU = mybir.AluOpType

    nc = bacc.Bacc("TRN2", target_bir_lowering=False, debug=False,
                   enable_asserts=False)

    xT = nc.dram_tensor("xT", [512, T_ * NB_], F16, kind="ExternalInput").ap()
    wih1 = nc.dram_tensor("wih1", [512, 12 * 128], F16, kind="ExternalInput").ap()
    whh1 = nc.dram_tensor("whh1", [512, 12 * 128], F16, kind="ExternalInput").ap()
    bias1 = nc.dram_tensor("bias1", [128, 12], F32, kind="ExternalInput").ap()
    biasn1 = nc.dram_tensor("biasn1", [128, 4 * NB_], F32, kind="ExternalInput").ap()
    wih2 = nc.dram_tensor("wih2", [512, 12 * 128], F16, kind="ExternalInput").ap()
    whh2 = nc.dram_tensor("whh2", [512, 12 * 128], F16, kind="ExternalInput").ap()
    bias2 = nc.dram_tensor("bias2", [128, 12], F32, kind="ExternalInput").ap()
    biasn2 = nc.dram_tensor("biasn2", [128, 4 * NB_], F32, kind="ExternalInput").ap()
    y = nc.dram_tensor("y", [128, T_ * 4 * NB_], F16, kind="ExternalOutput").ap()
    y4 = y.rearrange("p (t j b) -> p t j b", j=4, b=NB_)

    def emit_layer(tc, pools, wih_sb, whh_sb, bias_sb, biasn_sb,
                   get_rhs, pre_chunk, post_chunk):
        C = T_ // S_
        co = pools["state"].tile([128, (S_ + 1) * 4 * NB_], F16, tag="co")
        co4 = co.rearrange("p (t j b) -> p t j b", j=4, b=NB_)
        xp = pools["state"].tile([128, 12 * S_ * NB_], F32, tag="xp")
        xp4 = xp.rearrange("p (m t b) -> p m t b", m=12, b=NB_)
        nc.vector.memset(co4[:, 0, :, :], 0.0)

        for k in range(C):
            pre_chunk(k)
            for m in range(12):
                ps = pools["gemm_ps"].tile([128, S_ * NB_], F32, tag="gemm_ps")
                for j in range(4):
                    nc.tensor.matmul(
                        ps[:], wih_sb[:, (j * 12 + m) * 128:(j * 12 + m + 1) * 128],
                        get_rhs(k, j), start=(j == 0), stop=(j == 3))
                nc.vector.tensor_scalar_add(xp4[:, m, :, :], ps[:],
                                            bias_sb[:, m:m + 1])

            def step_body(iv):
                for u in range(U_):
                    i = iv + u if U_ > 1 else iv
                    hcur = co4[:, bass.ds(i, 1), :, :]
                    ps_rz = pools["ps_rz"].tile([128, 8 * NB_], F32, tag="ps_rz")
                    ps_n = pools["ps_n"].tile([128, 4 * NB_], F32, tag="ps_n")
                    for m in range(8):
                        for j in range(4):
                            nc.tensor.matmul(
                                ps_rz[:, m * NB_:(m + 1) * NB_],
                                whh_sb[:, (j * 12 + m) * 128:(j * 12 + m + 1) * 128],
                                hcur[:, 0, j, :], start=(j == 0), stop=(j == 3))
                    for m in range(8, 12):
                        for j in range(4):
                            nc.tensor.matmul(
                                ps_n[:, (m - 8) * NB_:(m - 7) * NB_],
                                whh_sb[:, (j * 12 + m) * 128:(j * 12 + m + 1) * 128],
                                hcur[:, 0, j, :], start=(j == 0), stop=(j == 3))
                    g = pools["gate"]
                    a_n = g.tile([128, 4 * NB_], F32, tag="a_n")
                    nc.vector.tensor_add(a_n[:], ps_n[:], biasn_sb[:])
                    s_rz = g.tile([128, 8 * NB_], F32, tag="s_rz")
                    nc.vector.tensor_add(s_rz[:], ps_rz[:],
                                         xp4[:, 0:8, bass.ds(i, 1), :])
                    rz = g.tile([128, 8 * NB_], F32, tag="rz")
                    nc.scalar.activation(rz[:], s_rz[:], AF.Sigmoid)
                    t1 = g.tile([128, 4 * NB_], F32, tag="t1")
                    nc.vector.tensor_mul(t1[:], rz[:, 0:4 * NB_], a_n[:])
                    t2 = g.tile([128, 4 * NB_], F32, tag="t2")
                    nc.vector.tensor_add(t2[:], t1[:],
                                         xp4[:, 8:12, bass.ds(i, 1), :])
                    nn_ = g.tile([128, 4 * NB_], F32, tag="nn")
                    nc.scalar.activation(nn_[:], t2[:], AF.Tanh)
                    d = g.tile([128, 4 * NB_], F32, tag="d")
                    nc.vector.tensor_sub(d[:], hcur[:, 0, :, :], nn_[:])
                    e = g.tile([128, 4 * NB_], F32, tag="e")
                    nc.vector.tensor_mul(e[:], rz[:, 4 * NB_:8 * NB_], d[:])
                    nc.vector.tensor_add(co4[:, bass.ds(i + 1, 1), :, :],
                                         nn_[:], e[:])

            if U_ >= S_:
                for i0 in range(S_):
                    step_body(i0)
            else:
                with tc.For_i(0, S_, U_) as iv:
                    step_body(iv)

            post_chunk(k, co4)
            if k != C - 1:
                nc.vector.tensor_copy(co4[:, 0, :, :], co4[:, S_, :, :])

    with tile.TileContext(nc) as tc, ExitStack() as ctx:
        pools = {
            "state": ctx.enter_context(tc.tile_pool(name="state", bufs=1)),
            "wpool": ctx.enter_context(tc.tile_pool(name="wpool", bufs=1)),
            "xin": ctx.enter_context(tc.tile_pool(name="xin", bufs=2)),
            "gemm_ps": ctx.enter_context(tc.tile_pool(name="gemm_ps", bufs=2, space="PSUM")),
            "ps_rz": ctx.enter_context(tc.tile_pool(name="ps_rz", bufs=2, space="PSUM")),
            "ps_n": ctx.enter_context(tc.tile_pool(name="ps_n", bufs=2, space="PSUM")),
            "gate": ctx.enter_context(tc.tile_pool(name="gate", bufs=2)),
            "gelu": ctx.enter_context(tc.tile_pool(name="gelu", bufs=2)),
        }

        def load_w(dram, name):
            t = pools["wpool"].tile([128, 4 * 12 * 128], F16, tag=name)
            for j in range(4):
                nc.sync.dma_start(t[:, j * 12 * 128:(j + 1) * 12 * 128],
                                  dram[j * 128:(j + 1) * 128, :])
            return t

        wih1_sb = load_w(wih1, "wih1")
        whh1_sb = load_w(whh1, "whh1")
        wih2_sb = load_w(wih2, "wih2")
        whh2_sb = load_w(whh2, "whh2")

        def load_b(dram, name, w):
            t = pools["wpool"].tile([128, w], F32, tag=name)
            nc.sync.dma_start(t[:], dram[:])
            return t

        bias1_sb = load_b(bias1, "bias1", 12)
        biasn1_sb = load_b(biasn1, "biasn1", 4 * NB_)
        bias2_sb = load_b(bias2, "bias2", 12)
        biasn2_sb = load_b(biasn2, "biasn2", 4 * NB_)

        mid = pools["state"].tile([128, T_ * 4 * NB_], F16, tag="mid")
        mid4 = mid.rearrange("p (t j b) -> p t j b", j=4, b=NB_)

        xin_tiles = {}

        def pre1(k):
            xs = pools["xin"].tile([128, 4 * S_ * NB_], F16, tag="xs")
            for j in range(4):
                nc.sync.dma_start(
                    xs[:, j * S_ * NB_:(j + 1) * S_ * NB_],
                    xT[j * 128:(j + 1) * 128, k * S_ * NB_:(k + 1) * S_ * NB_])
            xin_tiles[k] = xs

        def rhs1(k, j):
            return xin_tiles[k][:, j * S_ * NB_:(j + 1) * S_ * NB_]

        def post1(k, co4):
            src = co4[:, 1:S_ + 1, :, :]
            erf_t = pools["gelu"].tile([128, S_ * 4 * NB_], F32, tag="erf")
            nc.scalar.activation(erf_t[:], src, AF.Erf, scale=0.7071067811865476)
            xe = pools["gelu"].tile([128, S_ * 4 * NB_], F32, tag="xe")
            nc.vector.scalar_tensor_tensor(xe[:], src, 0.5, erf_t[:],
                                           op0=ALU.mult, op1=ALU.mult)
            nc.vector.scalar_tensor_tensor(
                mid4[:, k * S_:(k + 1) * S_, :, :], src, 0.5, xe[:],
                op0=ALU.mult, op1=ALU.add)

        emit_layer(tc, pools, wih1_sb, whh1_sb, bias1_sb, biasn1_sb,
                   rhs1, pre1, post1)

        def pre2(k):
            pass

        def rhs2(k, j):
            return mid4[:, k * S_:(k + 1) * S_, j, :]

        def post2(k, co4):
            nc.sync.dma_start(y4[:, k * S_:(k + 1) * S_, :, :],
                              co4[:, 1:S_ + 1, :, :])

        emit_layer(tc, pools, wih2_sb, whh2_sb, bias2_sb, biasn2_sb,
                   rhs2, pre2, post2)

    nc.compile()
    return nc


def _get_nc():
    key = (T, S, NB, U)
    if key not in _CACHE:
        _CACHE[key] = _build(T, S, NB, U)
    return _CACHE[key]


def _prep_core_inputs(x_slice, w_ih1, w_hh1, b_ih1, b_hh1,
                      w_ih2, w_hh2, b_ih2, b_hh2):
    def wstat(w):
        return np.ascontiguousarray(w.T).astype(np.float16)

    def biasv(b_ih, b_hh):
        b = b_ih.astype(np.float64).copy()
        b[:2 * H] += b_hh[:2 * H].astype(np.float64)
        return np.ascontiguousarray(b.reshape(12, 128).T).astype(np.float32)

    def biasn(b_hh):
        bn = b_hh[2 * H:].reshape(4, 128).T
        return np.ascontiguousarray(
            np.repeat(bn[:, :, None], NB, axis=2).reshape(128, 4 * NB)
        ).astype(np.float32)

    xT = np.ascontiguousarray(
        x_slice.transpose(2, 1, 0).reshape(512, T * NB)).astype(np.float16)
    return {
        "xT": xT,
        "wih1": wstat(w_ih1), "whh1": wstat(w_hh1),
        "bias1": biasv(b_ih1, b_hh1), "biasn1": biasn(b_hh1),
        "wih2": wstat(w_ih2), "whh2": wstat(w_hh2),
        "bias2": biasv(b_ih2, b_hh2), "biasn2": biasn(b_hh2),
    }


def kernel(x, w_ih1, w_hh1, b_ih1, b_hh1, w_ih2, w_hh2, b_ih2, b_hh2):
    from concourse import bass_utils

    x = np.asarray(x, dtype=np.float32)
    args = [np.asarray(a, dtype=np.float32) for a in
            (w_ih1, w_hh1, b_ih1, b_hh1, w_ih2, w_hh2, b_ih2, b_hh2)]

    nc = _get_nc()
    in_maps = [
        _prep_core_inputs(x[c * NB:(c + 1) * NB], *args)
        for c in range(N_CORES)
    ]
    res = bass_utils.run_bass_kernel_spmd(nc, in_maps,
                                          core_ids=list(range(N_CORES)))
    parts = []
    for c in range(N_CORES):
        yf = res.results[c]["y"].astype(np.float32).reshape(128, T, 4, NB)
        parts.append(np.ascontiguousarray(
            yf.transpose(3, 1, 2, 0).reshape(NB, T, 512)))
    return np.concatenate(parts, axis=0)



# revision 2
# speedup vs baseline: 2.9712x; 2.9712x over previous
"""nn_GRUBlock Trainium2 kernel: y = GRU2(gelu(GRU1(x))).

Self-contained: builds a Bass/Tile program, shards batch B=16 across 8
NeuronCores (B=2 per core), runs via run_bass_kernel_spmd, gathers the
full output.

Per-core program (v2 — layer-interleaved, static PE addressing):
  - the two GRU layers run as interleaved streams: while layer 1
    processes chunk k of the sequence, layer 2 processes chunk k-1.
    Each step, stream A's 48 hh-matmuls issue back-to-back, then stream
    B's; each stream's gate chain (DVE/ACT) hides under the other
    stream's matmul burst.
  - hidden state lives in a 2-slot ping-pong SBUF buffer so every
    matmul operand address is STATIC (no per-step register-offset APs on
    the PE queue); a cheap ACT copy archives h into the chunk history
    buffer for gelu / output DMA.
  - input projections as 128-tile GEMMs (moving N = S*NB cols)
  - matmul operands fp16 (fp32 PSUM accumulate + fp32 gates)
"""

from contextlib import ExitStack

import numpy as np

B, T, DIN, H = 16, 4096, 512, 512
N_CORES = 8
NB = B // N_CORES      # batch per core
S = 128                # chunk (steps)

_CACHE = {}


def _build(T_, S_, NB_):
    import concourse.bacc as bacc
    import concourse.bass as bass
    import concourse.tile as tile
    from concourse import mybir

    F32 = mybir.dt.float32
    F16 = mybir.dt.float16
    AF = mybir.ActivationFunctionType
    ALU = mybir.AluOpType

    nc = bacc.Bacc("TRN2", target_bir_lowering=False, debug=False,
                   enable_asserts=False)

    C = T_ // S_

    xT = nc.dram_tensor("xT", [512, T_ * NB_], F16, kind="ExternalInput").ap()
    wih1 = nc.dram_tensor("wih1", [512, 12 * 128], F16, kind="ExternalInput").ap()
    whh1 = nc.dram_tensor("whh1", [512, 12 * 128], F16, kind="ExternalInput").ap()
    bias1 = nc.dram_tensor("bias1", [128, 12], F32, kind="ExternalInput").ap()
    biasn1 = nc.dram_tensor("biasn1", [128, 4 * NB_], F32, kind="ExternalInput").ap()
    wih2 = nc.dram_tensor("wih2", [512, 12 * 128], F16, kind="ExternalInput").ap()
    whh2 = nc.dram_tensor("whh2", [512, 12 * 128], F16, kind="ExternalInput").ap()
    bias2 = nc.dram_tensor("bias2", [128, 12], F32, kind="ExternalInput").ap()
    biasn2 = nc.dram_tensor("biasn2", [128, 4 * NB_], F32, kind="ExternalInput").ap()
    y = nc.dram_tensor("y", [128, T_ * 4 * NB_], F16, kind="ExternalOutput").ap()
    y4 = y.rearrange("p (t j b) -> p t j b", j=4, b=NB_)

    with tile.TileContext(nc) as tc, ExitStack() as ctx:
        pools = {
            "state": ctx.enter_context(tc.tile_pool(name="state", bufs=1)),
            "wpool": ctx.enter_context(tc.tile_pool(name="wpool", bufs=1)),
            "xin": ctx.enter_context(tc.tile_pool(name="xin", bufs=2)),
            "xp": ctx.enter_context(tc.tile_pool(name="xp", bufs=2)),
            "co": ctx.enter_context(tc.tile_pool(name="co", bufs=2)),
            "mid": ctx.enter_context(tc.tile_pool(name="mid", bufs=2)),
            "gemm_ps": ctx.enter_context(tc.tile_pool(name="gemm_ps", bufs=2, space="PSUM")),
            "ps1": ctx.enter_context(tc.tile_pool(name="ps1", bufs=2, space="PSUM")),
            "ps2": ctx.enter_context(tc.tile_pool(name="ps2", bufs=2, space="PSUM")),
            "gate": ctx.enter_context(tc.tile_pool(name="gate", bufs=2)),
            "gelu": ctx.enter_context(tc.tile_pool(name="gelu", bufs=2)),
        }

        def load_w(dram, name):
            t = pools["wpool"].tile([128, 4 * 12 * 128], F16, tag=name)
            for j in range(4):
                nc.sync.dma_start(t[:, j * 12 * 128:(j + 1) * 12 * 128],
                                  dram[j * 128:(j + 1) * 128, :])
            return t

        wih1_sb = load_w(wih1, "wih1")
        whh1_sb = load_w(whh1, "whh1")
        wih2_sb = load_w(wih2, "wih2")
        whh2_sb = load_w(whh2, "whh2")

        def load_b(dram, name, w):
            t = pools["wpool"].tile([128, w], F32, tag=name)
            nc.sync.dma_start(t[:], dram[:])
            return t

        bias1_sb = load_b(bias1, "bias1", 12)
        biasn1_sb = load_b(biasn1, "biasn1", 4 * NB_)
        bias2_sb = load_b(bias2, "bias2", 12)
        biasn2_sb = load_b(biasn2, "biasn2", 4 * NB_)

        # ping-pong hidden state per stream: [128, slot(2), j(4), b]
        def h_state(name):
            t = pools["state"].tile([128, 2 * 4 * NB_], F16, tag=name)
            t4 = t.rearrange("p (s j b) -> p s j b", s=2, b=NB_)
            nc.vector.memset(t4[:, 0, :, :], 0.0)
            return t4

        h1 = h_state("h1")
        h2 = h_state("h2")

        xp_tiles = {}
        mid_tiles = {}
        co_tiles = {}

        def gemm(lyr, k, wih_sb, bias_sb, get_rhs):
            xp = pools["xp"].tile([128, 12 * S_ * NB_], F32, tag=f"xp{lyr}")
            xp4 = xp.rearrange("p (m t b) -> p m t b", m=12, b=NB_)
            for m in range(12):
                ps = pools["gemm_ps"].tile([128, S_ * NB_], F32, tag="gemm_ps")
                for j in range(4):
                    nc.tensor.matmul(
                        ps[:], wih_sb[:, (j * 12 + m) * 128:(j * 12 + m + 1) * 128],
                        get_rhs(j), start=(j == 0), stop=(j == 3))
                nc.vector.tensor_scalar_add(xp4[:, m, :, :], ps[:],
                                            bias_sb[:, m:m + 1])
            xp_tiles[(lyr, k)] = xp4

        def gemm1(k):
            xs = pools["xin"].tile([128, 4 * S_ * NB_], F16, tag="xs")
            for j in range(4):
                nc.sync.dma_start(
                    xs[:, j * S_ * NB_:(j + 1) * S_ * NB_],
                    xT[j * 128:(j + 1) * 128, k * S_ * NB_:(k + 1) * S_ * NB_])
            gemm(1, k, wih1_sb, bias1_sb,
                 lambda j: xs[:, j * S_ * NB_:(j + 1) * S_ * NB_])

        def gemm2(k):
            mid4 = mid_tiles[k]
            gemm(2, k, wih2_sb, bias2_sb, lambda j: mid4[:, :, j, :])

        def stream_step(lyr, h4, whh_sb, biasn_sb, xp4, co4, iv, u):
            i = iv + u if u is not None else iv
            ps = pools[f"ps{lyr}"].tile([128, 12 * NB_], F32, tag=f"ps{lyr}")
            for m in range(12):
                for j in range(4):
                    nc.tensor.matmul(
                        ps[:, m * NB_:(m + 1) * NB_],
                        whh_sb[:, (j * 12 + m) * 128:(j * 12 + m + 1) * 128],
                        h4[:, u, j, :], start=(j == 0), stop=(j == 3))
            g = pools["gate"]
            s_rz = g.tile([128, 8 * NB_], F32, tag=f"s_rz{lyr}")
            nc.vector.tensor_add(s_rz[:], ps[:, 0:8 * NB_],
                                 xp4[:, 0:8, bass.ds(i, 1), :])
            rz = g.tile([128, 8 * NB_], F32, tag=f"rz{lyr}")
            nc.scalar.activation(rz[:], s_rz[:], AF.Sigmoid)
            a_n = g.tile([128, 4 * NB_], F32, tag=f"a_n{lyr}")
            nc.vector.tensor_add(a_n[:], ps[:, 8 * NB_:12 * NB_], biasn_sb[:])
            t1 = g.tile([128, 4 * NB_], F32, tag=f"t1{lyr}")
            nc.vector.tensor_mul(t1[:], rz[:, 0:4 * NB_], a_n[:])
            t2 = g.tile([128, 4 * NB_], F32, tag=f"t2{lyr}")
            nc.vector.tensor_add(t2[:], t1[:], xp4[:, 8:12, bass.ds(i, 1), :])
            nn_ = g.tile([128, 4 * NB_], F32, tag=f"nn{lyr}")
            nc.scalar.activation(nn_[:], t2[:], AF.Tanh)
            d = g.tile([128, 4 * NB_], F32, tag=f"d{lyr}")
            nc.vector.tensor_sub(d[:], h4[:, u, :, :], nn_[:])
            e = g.tile([128, 4 * NB_], F32, tag=f"e{lyr}")
            nc.vector.tensor_mul(e[:], rz[:, 4 * NB_:8 * NB_], d[:])
            nc.vector.tensor_add(h4[:, 1 - u, :, :], nn_[:], e[:])
            nc.scalar.copy(co4[:, bass.ds(i, 1), :, :], h4[:, 1 - u, :, :])

        import concourse.bass as bass

        for k in range(C + 1):
            # chunk-k GEMMs for the two streams of this slot
            if k < C:
                gemm1(k)
                co1 = pools["co"].tile([128, S_ * 4 * NB_], F16, tag="co1")
                co1_4 = co1.rearrange("p (t j b) -> p t j b", j=4, b=NB_)
                co_tiles[(1, k)] = co1_4
            if k >= 1:
                gemm2(k - 1)
                co2 = pools["co"].tile([128, S_ * 4 * NB_], F16, tag="co2")
                co2_4 = co2.rearrange("p (t j b) -> p t j b", j=4, b=NB_)
                co_tiles[(2, k - 1)] = co2_4

            with tc.For_i(0, S_, 2) as iv:
                for u in range(2):
                    if k < C:
                        stream_step(1, h1, whh1_sb, biasn1_sb,
                                    xp_tiles[(1, k)], co_tiles[(1, k)], iv, u)
                    if k >= 1:
                        stream_step(2, h2, whh2_sb, biasn2_sb,
                                    xp_tiles[(2, k - 1)], co_tiles[(2, k - 1)],
                                    iv, u)

            if k < C:
                # gelu(h1 chunk k) -> mid chunk k
                src = co_tiles[(1, k)][:, :, :, :]
                mid = pools["mid"].tile([128, S_ * 4 * NB_], F16, tag="mid")
                mid4 = mid.rearrange("p (t j b) -> p t j b", j=4, b=NB_)
                erf_t = pools["gelu"].tile([128, S_ * 4 * NB_], F32, tag="erf")
                nc.scalar.activation(erf_t[:], src, AF.Erf,
                                     scale=0.7071067811865476)
                xe = pools["gelu"].tile([128, S_ * 4 * NB_], F32, tag="xe")
                nc.vector.scalar_tensor_tensor(xe[:], src, 0.5, erf_t[:],
                                               op0=ALU.mult, op1=ALU.mult)
                nc.vector.scalar_tensor_tensor(
                    mid4[:, :, :, :], src, 0.5, xe[:],
                    op0=ALU.mult, op1=ALU.add)
                mid_tiles[k] = mid4
            if k >= 1:
                nc.sync.dma_start(y4[:, (k - 1) * S_:k * S_, :, :],
                                  co_tiles[(2, k - 1)][:, :, :, :])

    nc.compile()
    return nc


def _get_nc():
    key = (T, S, NB)
    if key not in _CACHE:
        _CACHE[key] = _build(T, S, NB)
    return _CACHE[key]


def _prep_core_inputs(x_slice, w_ih1, w_hh1, b_ih1, b_hh1,
                      w_ih2, w_hh2, b_ih2, b_hh2):
    def wstat(w):
        return np.ascontiguousarray(w.T).astype(np.float16)

    def biasv(b_ih, b_hh):
        b = b_ih.astype(np.float64).copy()
        b[:2 * H] += b_hh[:2 * H].astype(np.float64)
        return np.ascontiguousarray(b.reshape(12, 128).T).astype(np.float32)

    def biasn(b_hh):
        bn = b_hh[2 * H:].reshape(4, 128).T
        return np.ascontiguousarray(
            np.repeat(bn[:, :, None], NB, axis=2).reshape(128, 4 * NB)
        ).astype(np.float32)

    xT = np.ascontiguousarray(
        x_slice.transpose(2, 1, 0).reshape(512, T * NB)).astype(np.float16)
    return {
        "xT": xT,
        "wih1": wstat(w_ih1), "whh1": wstat(w_hh1),
        "bias1": biasv(b_ih1, b_hh1), "biasn1": biasn(b_hh1),
        "wih2": wstat(w_ih2), "whh2": wstat(w_hh2),
        "bias2": biasv(b_ih2, b_hh2), "biasn2": biasn(b_hh2),
    }


def kernel(x, w_ih1, w_hh1, b_ih1, b_hh1, w_ih2, w_hh2, b_ih2, b_hh2):
    from concourse import bass_utils

    x = np.asarray(x, dtype=np.float32)
    args = [np.asarray(a, dtype=np.float32) for a in
            (w_ih1, w_hh1, b_ih1, b_hh1, w_ih2, w_hh2, b_ih2, b_hh2)]

    nc = _get_nc()
    in_maps = [
        _prep_core_inputs(x[c * NB:(c + 1) * NB], *args)
        for c in range(N_CORES)
    ]
    res = bass_utils.run_bass_kernel_spmd(nc, in_maps,
                                          core_ids=list(range(N_CORES)))
    parts = []
    for c in range(N_CORES):
        yf = res.results[c]["y"].astype(np.float32).reshape(128, T, 4, NB)
        parts.append(np.ascontiguousarray(
            yf.transpose(3, 1, 2, 0).reshape(NB, T, 512)))
    return np.concatenate(parts, axis=0)


# revision 4
# speedup vs baseline: 3.5155x; 1.1832x over previous
"""nn_GRUBlock Trainium2 kernel: y = GRU2(gelu(GRU1(x))).

Self-contained: builds a Bass/Tile program, shards batch B=16 across 8
NeuronCores (B=2 per core), runs via run_bass_kernel_spmd, gathers the
full output.

Per-core program (v3):
  - two GRU layers as interleaved streams (L1 chunk k with L2 chunk k-1)
    so each stream's gate chain hides under the other stream's matmuls
  - hidden state in a 2-slot ping-pong SBUF buffer: every PE operand
    address is static (no register-offset APs on the PE queue)
  - per step the xp slice + recurrent-bias adds are folded into PSUM via
    two identity matmuls (the xp slice is staged to a static tile by
    GpSimd ahead of time), shortening the serial gate chain to
    sigmoid -> t1 -> t2 -> tanh -> d -> e -> h'
  - matmul operands fp16 (fp32 PSUM accumulate + fp32 gates)
"""

from contextlib import ExitStack

import numpy as np

B, T, DIN, H = 16, 4096, 512, 512
N_CORES = 8
NB = B // N_CORES      # batch per core
S = 128                # chunk (steps)
U = 4                  # step unroll inside For_i (ping-pong period 2)

_CACHE = {}


def _build(T_, S_, NB_):
    import concourse.bacc as bacc
    import concourse.bass as bass
    import concourse.tile as tile
    from concourse import mybir

    F32 = mybir.dt.float32
    F16 = mybir.dt.float16
    AF = mybir.ActivationFunctionType
    ALU = mybir.AluOpType

    nc = bacc.Bacc("TRN2", target_bir_lowering=False, debug=False,
                   enable_asserts=False)

    C = T_ // S_

    xT = nc.dram_tensor("xT", [512, T_ * NB_], F16, kind="ExternalInput").ap()
    wih1 = nc.dram_tensor("wih1", [512, 12 * 128], F16, kind="ExternalInput").ap()
    whh1 = nc.dram_tensor("whh1", [512, 12 * 128], F16, kind="ExternalInput").ap()
    bias1 = nc.dram_tensor("bias1", [128, 12], F32, kind="ExternalInput").ap()
    biasn1 = nc.dram_tensor("biasn1", [128, 4 * NB_], F16, kind="ExternalInput").ap()
    wih2 = nc.dram_tensor("wih2", [512, 12 * 128], F16, kind="ExternalInput").ap()
    whh2 = nc.dram_tensor("whh2", [512, 12 * 128], F16, kind="ExternalInput").ap()
    bias2 = nc.dram_tensor("bias2", [128, 12], F32, kind="ExternalInput").ap()
    biasn2 = nc.dram_tensor("biasn2", [128, 4 * NB_], F16, kind="ExternalInput").ap()
    ident = nc.dram_tensor("ident", [128, 128], F16, kind="ExternalInput").ap()
    y = nc.dram_tensor("y", [128, T_ * 4 * NB_], F16, kind="ExternalOutput").ap()
    y4 = y.rearrange("p (t j b) -> p t j b", j=4, b=NB_)

    with tile.TileContext(nc) as tc, ExitStack() as ctx:
        pools = {
            "state": ctx.enter_context(tc.tile_pool(name="state", bufs=1)),
            "wpool": ctx.enter_context(tc.tile_pool(name="wpool", bufs=1)),
            "xin": ctx.enter_context(tc.tile_pool(name="xin", bufs=2)),
            "xp": ctx.enter_context(tc.tile_pool(name="xp", bufs=2)),
            "xst": ctx.enter_context(tc.tile_pool(name="xst", bufs=2)),
            "co": ctx.enter_context(tc.tile_pool(name="co", bufs=2)),
            "mid": ctx.enter_context(tc.tile_pool(name="mid", bufs=2)),
            "gemm_ps": ctx.enter_context(tc.tile_pool(name="gemm_ps", bufs=2, space="PSUM")),
            "psrz1": ctx.enter_context(tc.tile_pool(name="psrz1", bufs=1, space="PSUM")),
            "psn1": ctx.enter_context(tc.tile_pool(name="psn1", bufs=1, space="PSUM")),
            "psrz2": ctx.enter_context(tc.tile_pool(name="psrz2", bufs=1, space="PSUM")),
            "psn2": ctx.enter_context(tc.tile_pool(name="psn2", bufs=1, space="PSUM")),
            "gate": ctx.enter_context(tc.tile_pool(name="gate", bufs=2)),
            "gelu": ctx.enter_context(tc.tile_pool(name="gelu", bufs=2)),
        }

        def load_w(dram, name):
            t = pools["wpool"].tile([128, 4 * 12 * 128], F16, tag=name)
            for j in range(4):
                nc.sync.dma_start(t[:, j * 12 * 128:(j + 1) * 12 * 128],
                                  dram[j * 128:(j + 1) * 128, :])
            return t

        wih1_sb = load_w(wih1, "wih1")
        whh1_sb = load_w(whh1, "whh1")
        wih2_sb = load_w(wih2, "wih2")
        whh2_sb = load_w(whh2, "whh2")

        def load_b(dram, name, w, dt):
            t = pools["wpool"].tile([128, w], dt, tag=name)
            nc.sync.dma_start(t[:], dram[:])
            return t

        bias1_sb = load_b(bias1, "bias1", 12, F32)
        biasn1_sb = load_b(biasn1, "biasn1", 4 * NB_, F16)
        bias2_sb = load_b(bias2, "bias2", 12, F32)
        biasn2_sb = load_b(biasn2, "biasn2", 4 * NB_, F16)
        ident_sb = load_b(ident, "ident", 128, F16)

        # ping-pong hidden state per stream: [128, slot(2), j(4), b]
        def h_state(name):
            t = pools["state"].tile([128, 2 * 4 * NB_], F16, tag=name)
            t4 = t.rearrange("p (s j b) -> p s j b", s=2, b=NB_)
            nc.vector.memset(t4[:, 0, :, :], 0.0)
            return t4

        h1 = h_state("h1")
        h2 = h_state("h2")

        xp_tiles = {}
        mid_tiles = {}
        co_tiles = {}

        def gemm(lyr, k, wih_sb, bias_sb, get_rhs):
            xp = pools["xp"].tile([128, 12 * S_ * NB_], F16, tag=f"xp{lyr}")
            xp4 = xp.rearrange("p (m t b) -> p m t b", m=12, b=NB_)
            for m in range(12):
                ps = pools["gemm_ps"].tile([128, S_ * NB_], F32, tag="gemm_ps")
                for j in range(4):
                    nc.tensor.matmul(
                        ps[:], wih_sb[:, (j * 12 + m) * 128:(j * 12 + m + 1) * 128],
                        get_rhs(j), start=(j == 0), stop=(j == 3))
                nc.vector.tensor_scalar_add(xp4[:, m, :, :], ps[:],
                                            bias_sb[:, m:m + 1])
            xp_tiles[(lyr, k)] = xp4

        def gemm1(k):
            xs = pools["xin"].tile([128, 4 * S_ * NB_], F16, tag="xs")
            for j in range(4):
                nc.sync.dma_start(
                    xs[:, j * S_ * NB_:(j + 1) * S_ * NB_],
                    xT[j * 128:(j + 1) * 128, k * S_ * NB_:(k + 1) * S_ * NB_])
            gemm(1, k, wih1_sb, bias1_sb,
                 lambda j: xs[:, j * S_ * NB_:(j + 1) * S_ * NB_])

        def gemm2(k):
            mid4 = mid_tiles[k]
            gemm(2, k, wih2_sb, bias2_sb, lambda j: mid4[:, :, j, :])

        def stream_step(lyr, h4, whh_sb, biasn_sb, xp4, co4, iv, u):
            i = iv + u
            p = u % 2
            # stage this step's xp slice into a statically-addressed tile
            xst = pools["xst"].tile([128, 12 * NB_], F16, tag=f"xst{lyr}")
            nc.gpsimd.tensor_copy(xst[:], xp4[:, :, bass.ds(i, 1), :])
            ps_rz = pools[f"psrz{lyr}"].tile([128, 8 * NB_], F32, tag=f"psrz{lyr}")
            ps_n = pools[f"psn{lyr}"].tile([128, 4 * NB_], F32, tag=f"psn{lyr}")
            nc.tensor.matmul(ps_rz[:], ident_sb[:, :], xst[:, 0:8 * NB_],
                             start=True, stop=False, skip_group_check=True)
            for m in range(8):
                for j in range(4):
                    nc.tensor.matmul(
                        ps_rz[:, m * NB_:(m + 1) * NB_],
                        whh_sb[:, (j * 12 + m) * 128:(j * 12 + m + 1) * 128],
                        h4[:, p, j, :], start=False,
                        stop=(m == 7 and j == 3), skip_group_check=True)
            nc.tensor.matmul(ps_n[:], ident_sb[:, :], biasn_sb[:],
                             start=True, stop=False, skip_group_check=True)
            for m in range(8, 12):
                for j in range(4):
                    nc.tensor.matmul(
                        ps_n[:, (m - 8) * NB_:(m - 7) * NB_],
                        whh_sb[:, (j * 12 + m) * 128:(j * 12 + m + 1) * 128],
                        h4[:, p, j, :], start=False,
                        stop=(m == 11 and j == 3), skip_group_check=True)
            g = pools["gate"]
            rz = g.tile([128, 8 * NB_], F32, tag=f"rz{lyr}")
            nc.scalar.activation(rz[:], ps_rz[:], AF.Sigmoid)
            t1 = g.tile([128, 4 * NB_], F32, tag=f"t1{lyr}")
            nc.vector.tensor_mul(t1[:], rz[:, 0:4 * NB_], ps_n[:])
            t2 = g.tile([128, 4 * NB_], F32, tag=f"t2{lyr}")
            nc.vector.tensor_add(t2[:], t1[:], xst[:, 8 * NB_:12 * NB_])
            nn_ = g.tile([128, 4 * NB_], F32, tag=f"nn{lyr}")
            nc.scalar.activation(nn_[:], t2[:], AF.Tanh)
            d = g.tile([128, 4 * NB_], F32, tag=f"d{lyr}")
            nc.vector.tensor_sub(d[:], h4[:, p, :, :], nn_[:])
            e = g.tile([128, 4 * NB_], F32, tag=f"e{lyr}")
            nc.vector.tensor_mul(e[:], rz[:, 4 * NB_:8 * NB_], d[:])
            nc.vector.tensor_add(h4[:, 1 - p, :, :], nn_[:], e[:])
            nc.gpsimd.tensor_copy(co4[:, bass.ds(i, 1), :, :],
                                  h4[:, 1 - p, :, :])

        for k in range(C + 1):
            if k < C:
                gemm1(k)
                co1 = pools["co"].tile([128, S_ * 4 * NB_], F16, tag="co1")
                co1_4 = co1.rearrange("p (t j b) -> p t j b", j=4, b=NB_)
                co_tiles[(1, k)] = co1_4
            if k >= 1:
                gemm2(k - 1)
                co2 = pools["co"].tile([128, S_ * 4 * NB_], F16, tag="co2")
                co2_4 = co2.rearrange("p (t j b) -> p t j b", j=4, b=NB_)
                co_tiles[(2, k - 1)] = co2_4

            with tc.For_i(0, S_, U) as iv:
                for u in range(U):
                    if k < C:
                        stream_step(1, h1, whh1_sb, biasn1_sb,
                                    xp_tiles[(1, k)], co_tiles[(1, k)], iv, u)
                    if k >= 1:
                        stream_step(2, h2, whh2_sb, biasn2_sb,
                                    xp_tiles[(2, k - 1)], co_tiles[(2, k - 1)],
                                    iv, u)

            if k < C:
                src = co_tiles[(1, k)][:, :, :, :]
                mid = pools["mid"].tile([128, S_ * 4 * NB_], F16, tag="mid")
                mid4 = mid.rearrange("p (t j b) -> p t j b", j=4, b=NB_)
                erf_t = pools["gelu"].tile([128, S_ * 4 * NB_], F32, tag="erf")
                nc.scalar.activation(erf_t[:], src, AF.Erf,
                                     scale=0.7071067811865476)
                xe = pools["gelu"].tile([128, S_ * 4 * NB_], F32, tag="xe")
                nc.vector.scalar_tensor_tensor(xe[:], src, 0.5, erf_t[:],
                                               op0=ALU.mult, op1=ALU.mult)
                nc.vector.scalar_tensor_tensor(
                    mid4[:, :, :, :], src, 0.5, xe[:],
                    op0=ALU.mult, op1=ALU.add)
                mid_tiles[k] = mid4
            if k >= 1:
                nc.sync.dma_start(y4[:, (k - 1) * S_:k * S_, :, :],
                                  co_tiles[(2, k - 1)][:, :, :, :])

    nc.compile()
    return nc


def _get_nc():
    key = (T, S, NB, U)
    if key not in _CACHE:
        _CACHE[key] = _build(T, S, NB)
    return _CACHE[key]


def _prep_core_inputs(x_slice, w_ih1, w_hh1, b_ih1, b_hh1,
                      w_ih2, w_hh2, b_ih2, b_hh2):
    def wstat(w):
        return np.ascontiguousarray(w.T).astype(np.float16)

    def biasv(b_ih, b_hh):
        b = b_ih.astype(np.float64).copy()
        b[:2 * H] += b_hh[:2 * H].astype(np.float64)
        return np.ascontiguousarray(b.reshape(12, 128).T).astype(np.float32)

    def biasn(b_hh):
        bn = b_hh[2 * H:].reshape(4, 128).T
        return np.ascontiguousarray(
            np.repeat(bn[:, :, None], NB, axis=2).reshape(128, 4 * NB)
        ).astype(np.float16)

    xT = np.ascontiguousarray(
        x_slice.transpose(2, 1, 0).reshape(512, T * NB)).astype(np.float16)
    return {
        "xT": xT,
        "wih1": wstat(w_ih1), "whh1": wstat(w_hh1),
        "bias1": biasv(b_ih1, b_hh1), "biasn1": biasn(b_hh1),
        "wih2": wstat(w_ih2), "whh2": wstat(w_hh2),
        "bias2": biasv(b_ih2, b_hh2), "biasn2": biasn(b_hh2),
        "ident": np.eye(128, dtype=np.float16),
    }


def kernel(x, w_ih1, w_hh1, b_ih1, b_hh1, w_ih2, w_hh2, b_ih2, b_hh2):
    from concourse import bass_utils

    x = np.asarray(x, dtype=np.float32)
    args = [np.asarray(a, dtype=np.float32) for a in
            (w_ih1, w_hh1, b_ih1, b_hh1, w_ih2, w_hh2, b_ih2, b_hh2)]

    nc = _get_nc()
    in_maps = [
        _prep_core_inputs(x[c * NB:(c + 1) * NB], *args)
        for c in range(N_CORES)
    ]
    res = bass_utils.run_bass_kernel_spmd(nc, in_maps,
                                          core_ids=list(range(N_CORES)))
    parts = []
    for c in range(N_CORES):
        yf = res.results[c]["y"].astype(np.float32).reshape(128, T, 4, NB)
        parts.append(np.ascontiguousarray(
            yf.transpose(3, 1, 2, 0).reshape(NB, T, 512)))
    return np.concatenate(parts, axis=0)


# revision 7
# speedup vs baseline: 3.8034x; 1.0819x over previous
"""nn_GRUBlock Trainium2 kernel: y = GRU2(gelu(GRU1(x))).

Self-contained: builds a Bass/Tile program, shards batch B=16 across 8
NeuronCores (B=2 per core), runs via run_bass_kernel_spmd, gathers the
full output.

Per-core program (v3):
  - two GRU layers as interleaved streams (L1 chunk k with L2 chunk k-1)
    so each stream's gate chain hides under the other stream's matmuls
  - hidden state in a 2-slot ping-pong SBUF buffer: every PE operand
    address is static (no register-offset APs on the PE queue)
  - per step the xp slice + recurrent-bias adds are folded into PSUM via
    two identity matmuls (the xp slice is staged to a static tile by
    GpSimd ahead of time), shortening the serial gate chain to
    sigmoid -> t1 -> t2 -> tanh -> d -> e -> h'
  - matmul operands fp16 (fp32 PSUM accumulate + fp32 gates)
"""

from contextlib import ExitStack

import numpy as np

B, T, DIN, H = 16, 4096, 512, 512
N_CORES = 8
NB = B // N_CORES      # batch per core
S = 128                # chunk (steps)
U = 4                  # step unroll inside For_i (ping-pong period 2)

_CACHE = {}


def _build(T_, S_, NB_):
    import concourse.bacc as bacc
    import concourse.bass as bass
    import concourse.tile as tile
    from concourse import mybir

    F32 = mybir.dt.float32
    F16 = mybir.dt.float16
    AF = mybir.ActivationFunctionType
    ALU = mybir.AluOpType

    nc = bacc.Bacc("TRN2", target_bir_lowering=False, debug=False,
                   enable_asserts=False)

    C = T_ // S_

    xT = nc.dram_tensor("xT", [512, T_ * NB_], F16, kind="ExternalInput").ap()
    wih1 = nc.dram_tensor("wih1", [512, 12 * 128], F16, kind="ExternalInput").ap()
    whh1 = nc.dram_tensor("whh1", [512, 12 * 128], F16, kind="ExternalInput").ap()
    bias1 = nc.dram_tensor("bias1", [128, 12], F32, kind="ExternalInput").ap()
    biasn1 = nc.dram_tensor("biasn1", [128, 4 * NB_], F16, kind="ExternalInput").ap()
    wih2 = nc.dram_tensor("wih2", [512, 12 * 128], F16, kind="ExternalInput").ap()
    whh2 = nc.dram_tensor("whh2", [512, 12 * 128], F16, kind="ExternalInput").ap()
    bias2 = nc.dram_tensor("bias2", [128, 12], F32, kind="ExternalInput").ap()
    biasn2 = nc.dram_tensor("biasn2", [128, 4 * NB_], F16, kind="ExternalInput").ap()
    ident = nc.dram_tensor("ident", [128, 128], F16, kind="ExternalInput").ap()
    y = nc.dram_tensor("y", [128, T_ * 4 * NB_], F16, kind="ExternalOutput").ap()
    y4 = y.rearrange("p (t j b) -> p t j b", j=4, b=NB_)

    with tile.TileContext(nc) as tc, ExitStack() as ctx:
        pools = {
            "state": ctx.enter_context(tc.tile_pool(name="state", bufs=1)),
            "wpool": ctx.enter_context(tc.tile_pool(name="wpool", bufs=1)),
            "xin": ctx.enter_context(tc.tile_pool(name="xin", bufs=2)),
            "xp": ctx.enter_context(tc.tile_pool(name="xp", bufs=2)),
            "xst": ctx.enter_context(tc.tile_pool(name="xst", bufs=2)),
            "co": ctx.enter_context(tc.tile_pool(name="co", bufs=2)),
            "mid": ctx.enter_context(tc.tile_pool(name="mid", bufs=2)),
            "gemm_ps": ctx.enter_context(tc.tile_pool(name="gemm_ps", bufs=2, space="PSUM")),
            "psrz1": ctx.enter_context(tc.tile_pool(name="psrz1", bufs=1, space="PSUM")),
            "psn1": ctx.enter_context(tc.tile_pool(name="psn1", bufs=1, space="PSUM")),
            "psrz2": ctx.enter_context(tc.tile_pool(name="psrz2", bufs=1, space="PSUM")),
            "psn2": ctx.enter_context(tc.tile_pool(name="psn2", bufs=1, space="PSUM")),
            "gate": ctx.enter_context(tc.tile_pool(name="gate", bufs=2)),
            "gelu": ctx.enter_context(tc.tile_pool(name="gelu", bufs=2)),
        }

        def load_w(dram, name):
            t = pools["wpool"].tile([128, 4 * 12 * 128], F16, tag=name)
            for j in range(4):
                nc.sync.dma_start(t[:, j * 12 * 128:(j + 1) * 12 * 128],
                                  dram[j * 128:(j + 1) * 128, :])
            return t

        wih1_sb = load_w(wih1, "wih1")
        whh1_sb = load_w(whh1, "whh1")
        wih2_sb = load_w(wih2, "wih2")
        whh2_sb = load_w(whh2, "whh2")

        def load_b(dram, name, w, dt):
            t = pools["wpool"].tile([128, w], dt, tag=name)
            nc.sync.dma_start(t[:], dram[:])
            return t

        bias1_sb = load_b(bias1, "bias1", 12, F32)
        biasn1_sb = load_b(biasn1, "biasn1", 4 * NB_, F16)
        bias2_sb = load_b(bias2, "bias2", 12, F32)
        biasn2_sb = load_b(biasn2, "biasn2", 4 * NB_, F16)
        ident_sb = load_b(ident, "ident", 128, F16)

        # ping-pong hidden state per stream: [128, slot(2), j(4), b]
        def h_state(name):
            t = pools["state"].tile([128, 2 * 4 * NB_], F16, tag=name)
            t4 = t.rearrange("p (s j b) -> p s j b", s=2, b=NB_)
            nc.vector.memset(t4[:, 0, :, :], 0.0)
            return t4

        h1 = h_state("h1")
        h2 = h_state("h2")

        xp_tiles = {}
        mid_tiles = {}
        co_tiles = {}

        H2 = S_ // 2

        def gemm(lyr, k, wih_sb, bias_sb, get_rhs):
            xp = pools["xp"].tile([128, 12 * S_ * NB_], F16, tag=f"xp{lyr}")
            xp4 = xp.rearrange("p (m t b) -> p m t b", m=12, b=NB_)
            for hf in range(2):
                t0, t1_ = hf * H2, (hf + 1) * H2
                for m in range(12):
                    ps = pools["gemm_ps"].tile([128, H2 * NB_], F32,
                                               tag="gemm_ps")
                    for j in range(4):
                        nc.tensor.matmul(
                            ps[:],
                            wih_sb[:, (j * 12 + m) * 128:(j * 12 + m + 1) * 128],
                            get_rhs(j, t0, t1_), start=(j == 0), stop=(j == 3))
                    nc.vector.tensor_scalar_add(xp4[:, m, t0:t1_, :], ps[:],
                                                bias_sb[:, m:m + 1])
            xp_tiles[(lyr, k)] = xp4

        def gemm1(k):
            xs = pools["xin"].tile([128, 4 * S_ * NB_], F16, tag="xs")
            for j in range(4):
                nc.sync.dma_start(
                    xs[:, j * S_ * NB_:(j + 1) * S_ * NB_],
                    xT[j * 128:(j + 1) * 128, k * S_ * NB_:(k + 1) * S_ * NB_])
            gemm(1, k, wih1_sb, bias1_sb,
                 lambda j, t0, t1_: xs[:, j * S_ * NB_ + t0 * NB_:
                                       j * S_ * NB_ + t1_ * NB_])

        def gemm2(k):
            mid4 = mid_tiles[k]
            gemm(2, k, wih2_sb, bias2_sb,
                 lambda j, t0, t1_: mid4[:, t0:t1_, j, :])

        def stream_step(lyr, h4, whh_sb, biasn_sb, xp4, co4, iv, u):
            i = iv + u
            p = u % 2
            # stage this step's xp slice into a statically-addressed tile
            xst = pools["xst"].tile([128, 12 * NB_], F16, tag=f"xst{lyr}")
            nc.gpsimd.tensor_copy(xst[:], xp4[:, :, bass.ds(i, 1), :])
            ps_rz = pools[f"psrz{lyr}"].tile([128, 8 * NB_], F32, tag=f"psrz{lyr}")
            ps_n = pools[f"psn{lyr}"].tile([128, 4 * NB_], F32, tag=f"psn{lyr}")
            nc.tensor.matmul(ps_rz[:], ident_sb[:, :], xst[:, 0:8 * NB_],
                             start=True, stop=False, skip_group_check=True)
            for m in range(8):
                for j in range(4):
                    nc.tensor.matmul(
                        ps_rz[:, m * NB_:(m + 1) * NB_],
                        whh_sb[:, (j * 12 + m) * 128:(j * 12 + m + 1) * 128],
                        h4[:, p, j, :], start=False,
                        stop=(m == 7 and j == 3), skip_group_check=True)
            nc.tensor.matmul(ps_n[:], ident_sb[:, :], biasn_sb[:],
                             start=True, stop=False, skip_group_check=True)
            for m in range(8, 12):
                for j in range(4):
                    nc.tensor.matmul(
                        ps_n[:, (m - 8) * NB_:(m - 7) * NB_],
                        whh_sb[:, (j * 12 + m) * 128:(j * 12 + m + 1) * 128],
                        h4[:, p, j, :], start=False,
                        stop=(m == 11 and j == 3), skip_group_check=True)
            g = pools["gate"]
            rz = g.tile([128, 8 * NB_], F32, tag=f"rz{lyr}")
            nc.scalar.activation(rz[:], ps_rz[:], AF.Sigmoid)
            # critical chain: sigmoid -> t1 -> t2 -> tanh -> m1 -> h'
            # (w = 1-z and q = z*h run on V while tanh is on ACT)
            t1 = g.tile([128, 4 * NB_], F32, tag=f"t1{lyr}")
            nc.vector.tensor_mul(t1[:], rz[:, 0:4 * NB_], ps_n[:])
            t2 = g.tile([128, 4 * NB_], F32, tag=f"t2{lyr}")
            nc.vector.tensor_add(t2[:], t1[:], xst[:, 8 * NB_:12 * NB_])
            nn_ = g.tile([128, 4 * NB_], F32, tag=f"nn{lyr}")
            nc.scalar.activation(nn_[:], t2[:], AF.Tanh)
            w = g.tile([128, 4 * NB_], F32, tag=f"w{lyr}")
            nc.vector.tensor_scalar(w[:], rz[:, 4 * NB_:8 * NB_], -1.0, 1.0,
                                    op0=ALU.mult, op1=ALU.add)
            q = g.tile([128, 4 * NB_], F32, tag=f"q{lyr}")
            nc.vector.tensor_mul(q[:], rz[:, 4 * NB_:8 * NB_], h4[:, p, :, :])
            m1 = g.tile([128, 4 * NB_], F32, tag=f"m1{lyr}")
            nc.vector.tensor_mul(m1[:], nn_[:], w[:])
            nc.vector.tensor_add(h4[:, 1 - p, :, :], m1[:], q[:])
            nc.scalar.copy(co4[:, bass.ds(i, 1), :, :], h4[:, 1 - p, :, :])

        for k in range(C + 1):
            if k < C:
                gemm1(k)
                co1 = pools["co"].tile([128, S_ * 4 * NB_], F16, tag="co1")
                co1_4 = co1.rearrange("p (t j b) -> p t j b", j=4, b=NB_)
                co_tiles[(1, k)] = co1_4
            if k >= 1:
                gemm2(k - 1)
                co2 = pools["co"].tile([128, S_ * 4 * NB_], F16, tag="co2")
                co2_4 = co2.rearrange("p (t j b) -> p t j b", j=4, b=NB_)
                co_tiles[(2, k - 1)] = co2_4

            with tc.For_i(0, S_, U) as iv:
                for u in range(U):
                    if k < C:
                        stream_step(1, h1, whh1_sb, biasn1_sb,
                                    xp_tiles[(1, k)], co_tiles[(1, k)], iv, u)
                    if k >= 1:
                        stream_step(2, h2, whh2_sb, biasn2_sb,
                                    xp_tiles[(2, k - 1)], co_tiles[(2, k - 1)],
                                    iv, u)

            if k < C:
                mid = pools["mid"].tile([128, S_ * 4 * NB_], F16, tag="mid")
                mid4 = mid.rearrange("p (t j b) -> p t j b", j=4, b=NB_)
                for hf in range(2):
                    t0, t1_ = hf * H2, (hf + 1) * H2
                    src = co_tiles[(1, k)][:, t0:t1_, :, :]
                    erf_t = pools["gelu"].tile([128, H2 * 4 * NB_], F32,
                                               tag="erf")
                    nc.scalar.activation(erf_t[:], src, AF.Erf,
                                         scale=0.7071067811865476)
                    xe = pools["gelu"].tile([128, H2 * 4 * NB_], F32, tag="xe")
                    nc.vector.scalar_tensor_tensor(xe[:], src, 0.5, erf_t[:],
                                                   op0=ALU.mult, op1=ALU.mult)
                    nc.vector.scalar_tensor_tensor(
                        mid4[:, t0:t1_, :, :], src, 0.5, xe[:],
                        op0=ALU.mult, op1=ALU.add)
                mid_tiles[k] = mid4
            if k >= 1:
                nc.sync.dma_start(y4[:, (k - 1) * S_:k * S_, :, :],
                                  co_tiles[(2, k - 1)][:, :, :, :])

    nc.compile()
    return nc


def _get_nc():
    key = (T, S, NB, U)
    if key not in _CACHE:
        _CACHE[key] = _build(T, S, NB)
    return _CACHE[key]


def _prep_core_inputs(x_slice, w_ih1, w_hh1, b_ih1, b_hh1,
                      w_ih2, w_hh2, b_ih2, b_hh2):
    def wstat(w):
        return np.ascontiguousarray(w.T).astype(np.float16)

    def biasv(b_ih, b_hh):
        b = b_ih.astype(np.float64).copy()
        b[:2 * H] += b_hh[:2 * H].astype(np.float64)
        return np.ascontiguousarray(b.reshape(12, 128).T).astype(np.float32)

    def biasn(b_hh):
        bn = b_hh[2 * H:].reshape(4, 128).T
        return np.ascontiguousarray(
            np.repeat(bn[:, :, None], NB, axis=2).reshape(128, 4 * NB)
        ).astype(np.float16)

    xT = np.ascontiguousarray(
        x_slice.transpose(2, 1, 0).reshape(512, T * NB)).astype(np.float16)
    return {
        "xT": xT,
        "wih1": wstat(w_ih1), "whh1": wstat(w_hh1),
        "bias1": biasv(b_ih1, b_hh1), "biasn1": biasn(b_hh1),
        "wih2": wstat(w_ih2), "whh2": wstat(w_hh2),
        "bias2": biasv(b_ih2, b_hh2), "biasn2": biasn(b_hh2),
        "ident": np.eye(128, dtype=np.float16),
    }


def kernel(x, w_ih1, w_hh1, b_ih1, b_hh1, w_ih2, w_hh2, b_ih2, b_hh2):
    from concourse import bass_utils

    x = np.asarray(x, dtype=np.float32)
    args = [np.asarray(a, dtype=np.float32) for a in
            (w_ih1, w_hh1, b_ih1, b_hh1, w_ih2, w_hh2, b_ih2, b_hh2)]

    nc = _get_nc()
    in_maps = [
        _prep_core_inputs(x[c * NB:(c + 1) * NB], *args)
        for c in range(N_CORES)
    ]
    res = bass_utils.run_bass_kernel_spmd(nc, in_maps,
                                          core_ids=list(range(N_CORES)))
    parts = []
    for c in range(N_CORES):
        yf = res.results[c]["y"].astype(np.float32).reshape(128, T, 4, NB)
        parts.append(np.ascontiguousarray(
            yf.transpose(3, 1, 2, 0).reshape(NB, T, 512)))
    return np.concatenate(parts, axis=0)
